# revision 6
# baseline (speedup 1.0000x reference)
"""CrossFusion block kernel for 8 Trainium2 NeuronCores.

Strategy: pure data parallelism over the batch dimension (dim 0 of
hidden_states / past_key_values), weights replicated — matching the
no-cross-example structure of the module. The full block (LN1 -> q/trs/gelu,
k/v proj, raw-reshape head mix, dual softmax attention weighting, trs2,
out proj, residual, LN2, quickgelu MLP, residual) is compiled once with XLA
for a per-core batch shard of 64 examples and executed on all 8 cores in
parallel via shard_map.

A numpy path computes example 0 independently to self-check the device
result; on any failure (or numeric mismatch) the kernel falls back to the
host computation so the returned output is always correct.
"""

import numpy as np

B, TGT, SRC = 512, 49, 40
E, H = 768, 12
HD = E // H
FF = 3072
EPS = 1e-5
NCORES = 8


def _block_np(hidden_states, past_key_values, ln1_g, ln1_b, q_w, q_b, k_w, k_b,
              v_w, v_b, wqk_w, wqk_b, wk_w, wk_b, wb_w, wb_b, wm_w, wm_b,
              trs_w, trs_b, trs2_w, trs2_b, out_w, out_b, ln2_g, ln2_b,
              fc1_w, fc1_b, fc2_w, fc2_b, xp):
    """Reference math on module xp (numpy or jax.numpy), batch-shape agnostic."""
    Bs = hidden_states.shape[0]

    def ln(x, g, b):
        mu = xp.mean(x, axis=-1, keepdims=True)
        var = xp.var(x, axis=-1, keepdims=True)
        return (x - mu) / xp.sqrt(var + EPS) * g + b

    def erf(x):
        if xp is np:
            from scipy.special import erf as _e
            return _e(x)
        import jax
        return jax.scipy.special.erf(x)

    def mm(a, w):
        if xp is np:
            return a @ w
        import jax.numpy as jnp
        return jnp.matmul(a.astype(jnp.bfloat16), w.astype(jnp.bfloat16),
                          preferred_element_type=jnp.float32)

    residual = hidden_states
    x = ln(hidden_states, ln1_g, ln1_b)

    q = mm(x, q_w) + q_b                               # [B,49,E]
    # trs over seq dim without materializing the [B,E,49] transpose
    if xp is np:
        q = xp.einsum("bte,ts->bse", q, trs_w) + trs_b[None, :, None]  # [B,40,E]
    else:
        import jax.numpy as jnp
        q = jnp.einsum("bte,ts->bse", q.astype(jnp.bfloat16),
                       trs_w.astype(jnp.bfloat16),
                       preferred_element_type=jnp.float32) + trs_b[None, :, None]
    q = 0.5 * q * (1.0 + erf(q / np.sqrt(2.0).astype(np.float32)))  # exact gelu

    k = mm(past_key_values, k_w) + k_b                    # [B,40,E]
    v = mm(past_key_values, v_w) + v_b

    # faithful raw reshape to [B, H, SRC, HD] == contiguous regroup
    q = q.reshape(Bs, H, SRC, HD)
    k = k.reshape(Bs, H, SRC, HD)
    v = v.reshape(Bs, H, SRC, HD)

    Bk = (mm(q, wqk_w) + wqk_b) * (mm(k, wk_w) + wk_b)  # [B,H,40,HD]
    b = Bk @ wb_w + wb_b                               # [B,H,40,1]
    m = xp.einsum("bhsd,s->bhd", Bk, wm_w[:, 0])[:, :, None, :] + wm_b  # [B,H,1,HD]

    def softmax(z, axis):
        z = z - xp.max(z, axis=axis, keepdims=True)
        ez = xp.exp(z)
        return ez / xp.sum(ez, axis=axis, keepdims=True)

    attn_w = softmax(b, axis=-2)
    attn_w2 = softmax(m, axis=-1) - np.float32(0.05)

    out = (attn_w + attn_w2) * v                       # [B,H,40,HD]

    if xp is np:
        out = xp.einsum("bhsd,st->bhtd", out, trs2_w) + trs2_b[None, None, :, None]
    else:
        import jax.numpy as jnp
        out = jnp.einsum("bhsd,st->bhtd", out.astype(jnp.bfloat16),
                         trs2_w.astype(jnp.bfloat16),
                         preferred_element_type=jnp.float32) \
            + trs2_b[None, None, :, None]
    out = xp.swapaxes(out, 1, 2).reshape(Bs, TGT, E)
    out = mm(out, out_w) + out_b

    hs = residual + out
    x = ln(hs, ln2_g, ln2_b)
    h1 = mm(x, fc1_w) + fc1_b
    h1 = h1 * (1.0 / (1.0 + xp.exp(-1.702 * h1)))      # quick_gelu
    x = mm(h1, fc2_w) + fc2_b
    return hs + x


_ARGS = ["hidden_states", "past_key_values", "ln1_g", "ln1_b", "q_w", "q_b",
         "k_w", "k_b", "v_w", "v_b", "wqk_w", "wqk_b", "wk_w", "wk_b",
         "wb_w", "wb_b", "wm_w", "wm_b", "trs_w", "trs_b", "trs2_w", "trs2_b",
         "out_w", "out_b", "ln2_g", "ln2_b", "fc1_w", "fc1_b", "fc2_w", "fc2_b"]

_cached = {}


def _device_fn():
    """Build (once) the jitted 8-core data-parallel executable."""
    if "fn" in _cached:
        return _cached["fn"]
    import jax
    from jax.sharding import Mesh, PartitionSpec as P
    from jax.experimental.shard_map import shard_map

    devices = jax.devices()[:NCORES]
    assert len(devices) == NCORES
    mesh = Mesh(np.asarray(devices), ("core",))
    in_specs = tuple([P("core"), P("core")] + [P()] * (len(_ARGS) - 2))

    def body(*args):
        return _block_np(*args, xp=jax.numpy)

    fn = jax.jit(shard_map(body, mesh=mesh, in_specs=in_specs,
                           out_specs=P("core"), check_rep=False))
    _cached["fn"] = fn
    return fn


def kernel(**inputs) -> np.ndarray:
    args = [np.asarray(inputs[n], dtype=np.float32) for n in _ARGS]

    result = None
    try:
        fn = _device_fn()
        out = fn(*args)
        result = np.asarray(out, dtype=np.float32)
        if not _cached.get("checked"):
            # host check value: example 0 only (first call only)
            chk = _block_np(*([args[0][:1], args[1][:1]] + args[2:]), xp=np)
            scale = max(float(np.abs(chk).max()), 1e-6)
            rel = float(np.abs(result[:1] - chk).max()) / scale
            if not np.isfinite(rel) or rel > 1.5e-2:
                result = None
            else:
                _cached["checked"] = True
    except Exception:
        result = None

    if result is None:
        # host fallback: always correct
        result = np.asarray(_block_np(*args, xp=np), dtype=np.float32)
    return result


if __name__ == "__main__":
    rng = np.random.default_rng(0)
    ins = {n: rng.standard_normal((1,), dtype=np.float32) for n in _ARGS}
    print("module import ok")



# revision 11
# speedup vs baseline: 31.1907x; 31.1907x over previous
"""CrossFusion block kernel for 8 Trainium2 NeuronCores.

Strategy: pure data parallelism over the batch dimension (dim 0 of
hidden_states / past_key_values), weights replicated — matching the
no-cross-example structure of the module. The full block (LN1 -> q/trs/gelu,
k/v proj, raw-reshape head mix, dual softmax attention weighting, trs2,
out proj, residual, LN2, quickgelu MLP, residual) is compiled once with XLA
for a per-core batch shard of 64 examples and executed on all 8 cores in
parallel via shard_map.

A numpy path computes example 0 independently to self-check the device
result; on any failure (or numeric mismatch) the kernel falls back to the
host computation so the returned output is always correct.
"""

import numpy as np

B, TGT, SRC = 512, 49, 40
E, H = 768, 12
HD = E // H
FF = 3072
EPS = 1e-5
NCORES = 8


def _block_np(hidden_states, past_key_values, ln1_g, ln1_b, q_w, q_b, k_w, k_b,
              v_w, v_b, wqk_w, wqk_b, wk_w, wk_b, wb_w, wb_b, wm_w, wm_b,
              trs_w, trs_b, trs2_w, trs2_b, out_w, out_b, ln2_g, ln2_b,
              fc1_w, fc1_b, fc2_w, fc2_b, xp):
    """Reference math on module xp (numpy or jax.numpy), batch-shape agnostic."""
    Bs = hidden_states.shape[0]

    def ln(x, g, b):
        mu = xp.mean(x, axis=-1, keepdims=True)
        var = xp.var(x, axis=-1, keepdims=True)
        return (x - mu) / xp.sqrt(var + EPS) * g + b

    def erf(x):
        if xp is np:
            from scipy.special import erf as _e
            return _e(x)
        import jax
        return jax.scipy.special.erf(x)

    def mm(a, w):
        return a @ w

    residual = hidden_states
    x = ln(hidden_states, ln1_g, ln1_b)

    q = mm(x, q_w) + q_b                               # [B,49,E]
    # trs over seq dim without materializing the [B,E,49] transpose
    q = xp.einsum("bte,ts->bse", q, trs_w) + trs_b[None, :, None]  # [B,40,E]
    q = 0.5 * q * (1.0 + erf(q / np.sqrt(2.0).astype(np.float32)))  # exact gelu

    k = mm(past_key_values, k_w) + k_b                    # [B,40,E]
    v = mm(past_key_values, v_w) + v_b

    # faithful raw reshape to [B, H, SRC, HD] == contiguous regroup
    q = q.reshape(Bs, H, SRC, HD)
    k = k.reshape(Bs, H, SRC, HD)
    v = v.reshape(Bs, H, SRC, HD)

    Bk = (mm(q, wqk_w) + wqk_b) * (mm(k, wk_w) + wk_b)  # [B,H,40,HD]
    b = Bk @ wb_w + wb_b                               # [B,H,40,1]
    m = xp.einsum("bhsd,s->bhd", Bk, wm_w[:, 0])[:, :, None, :] + wm_b  # [B,H,1,HD]

    def softmax(z, axis):
        z = z - xp.max(z, axis=axis, keepdims=True)
        ez = xp.exp(z)
        return ez / xp.sum(ez, axis=axis, keepdims=True)

    attn_w = softmax(b, axis=-2)
    attn_w2 = softmax(m, axis=-1) - np.float32(0.05)

    out = (attn_w + attn_w2) * v                       # [B,H,40,HD]

    out = xp.einsum("bhsd,st->bhtd", out, trs2_w) + trs2_b[None, None, :, None]
    out = xp.swapaxes(out, 1, 2).reshape(Bs, TGT, E)
    out = mm(out, out_w) + out_b

    hs = residual + out
    x = ln(hs, ln2_g, ln2_b)
    h1 = mm(x, fc1_w) + fc1_b
    h1 = h1 * (1.0 / (1.0 + xp.exp(-1.702 * h1)))      # quick_gelu
    x = mm(h1, fc2_w) + fc2_b
    return hs + x


def _block_jax(hidden_states, past_key_values, ln1_g, ln1_b, q_w, q_b, k_w, k_b,
               v_w, v_b, wqk_w, wqk_b, wk_w, wk_b, wb_w, wb_b, wm_w, wm_b,
               trs_w, trs_b, trs2_w, trs2_b, out_w, out_b, ln2_g, ln2_b,
               fc1_w, fc1_b, fc2_w, fc2_b):
    """XLA-friendly reformulation: 2D GEMMs, no per-head batched matmuls,
    no materialized [B,E,49] transposes."""
    import jax
    import jax.numpy as jnp

    Bs = hidden_states.shape[0]
    Rt, Rs = Bs * TGT, Bs * SRC

    def ln(x, g, b):
        mu = jnp.mean(x, axis=-1, keepdims=True)
        var = jnp.var(x, axis=-1, keepdims=True)
        return (x - mu) * jax.lax.rsqrt(var + EPS) * g + b

    residual = hidden_states                               # [B,49,E]
    x = ln(hidden_states, ln1_g, ln1_b)

    q = (x.reshape(Rt, E) @ q_w + q_b).reshape(Bs, TGT, E)
    # trs over the token dim (49 -> 40)
    q = jnp.einsum("bte,ts->bse", q, trs_w) + trs_b[None, :, None]
    q = 0.5 * q * (1.0 + jax.scipy.special.erf(q * np.float32(1.0 / np.sqrt(2.0))))

    pk2 = past_key_values.reshape(Rs, E)
    k = pk2 @ k_w + k_b                                    # [Rs,E]
    v = pk2 @ v_w + v_b

    # faithful raw reshape: [B,40,768] == [B*480, 64] contiguous
    q4 = q.reshape(Bs * H * SRC, HD)
    k4 = k.reshape(Bs * H * SRC, HD)
    v4 = v.reshape(Bs, H, SRC, HD)

    BB = (q4 @ wqk_w + wqk_b) * (k4 @ wk_w + wk_b)         # [B*480, 64]
    bb = (BB @ wb_w + wb_b).reshape(Bs, H, SRC)            # [B,H,40]
    BB4 = BB.reshape(Bs, H, SRC, HD)
    m = (BB4 * wm_w[:, 0][None, None, :, None]).sum(-2) + wm_b  # [B,H,64]

    attn_w = jax.nn.softmax(bb, axis=-1)                   # over 40
    attn_w2 = jax.nn.softmax(m, axis=-1) - np.float32(0.05)  # over 64

    out = (attn_w[..., :, None] + attn_w2[..., None, :]) * v4   # [B,H,40,64]

    out = jnp.einsum("bhsd,st->bhtd", out, trs2_w) + trs2_b[None, None, :, None]
    out = out.transpose(0, 2, 1, 3).reshape(Rt, E)
    out = out @ out_w + out_b

    hs = residual.reshape(Rt, E) + out
    x = ln(hs, ln2_g, ln2_b)
    h1 = x @ fc1_w + fc1_b
    h1 = h1 * jax.nn.sigmoid(np.float32(1.702) * h1)
    y = h1 @ fc2_w + fc2_b
    return (hs + y).reshape(Bs, TGT, E)


_ARGS = ["hidden_states", "past_key_values", "ln1_g", "ln1_b", "q_w", "q_b",
         "k_w", "k_b", "v_w", "v_b", "wqk_w", "wqk_b", "wk_w", "wk_b",
         "wb_w", "wb_b", "wm_w", "wm_b", "trs_w", "trs_b", "trs2_w", "trs2_b",
         "out_w", "out_b", "ln2_g", "ln2_b", "fc1_w", "fc1_b", "fc2_w", "fc2_b"]

_cached = {}


def _device_fn():
    """Build (once) the jitted 8-core data-parallel executable."""
    if "fn" in _cached:
        return _cached["fn"]
    import jax
    from jax.sharding import Mesh, PartitionSpec as P
    from jax.experimental.shard_map import shard_map

    devices = jax.devices()[:NCORES]
    assert len(devices) == NCORES
    mesh = Mesh(np.asarray(devices), ("core",))
    in_specs = tuple([P("core"), P("core")] + [P()] * (len(_ARGS) - 2))

    def body(*args):
        return _block_jax(*args)

    fn = jax.jit(shard_map(body, mesh=mesh, in_specs=in_specs,
                           out_specs=P("core"), check_rep=False))
    _cached["fn"] = fn
    return fn


def kernel(**inputs) -> np.ndarray:
    args = [np.asarray(inputs[n], dtype=np.float32) for n in _ARGS]

    result = None
    try:
        fn = _device_fn()
        out = fn(*args)
        result = np.asarray(out, dtype=np.float32)
        if not _cached.get("checked"):
            # host check value: example 0 only (first call only)
            chk = _block_np(*([args[0][:1], args[1][:1]] + args[2:]), xp=np)
            scale = max(float(np.abs(chk).max()), 1e-6)
            rel = float(np.abs(result[:1] - chk).max()) / scale
            if not np.isfinite(rel) or rel > 1.5e-2:
                result = None
            else:
                _cached["checked"] = True
    except Exception:
        result = None

    if result is None:
        # host fallback: always correct
        result = np.asarray(_block_np(*args, xp=np), dtype=np.float32)
    return result


if __name__ == "__main__":
    rng = np.random.default_rng(0)
    ins = {n: rng.standard_normal((1,), dtype=np.float32) for n in _ARGS}
    print("module import ok")



# revision 15
# speedup vs baseline: 40.8738x; 1.3105x over previous
"""CrossFusion block kernel for 8 Trainium2 NeuronCores.

Strategy: pure data parallelism over the batch dimension (dim 0 of
hidden_states / past_key_values), weights replicated — matching the
no-cross-example structure of the module. The full block (LN1 -> q/trs/gelu,
k/v proj, raw-reshape head mix, dual softmax attention weighting, trs2,
out proj, residual, LN2, quickgelu MLP, residual) is compiled once with XLA
for a per-core batch shard of 64 examples and executed on all 8 cores in
parallel via shard_map.

A numpy path computes example 0 independently to self-check the device
result; on any failure (or numeric mismatch) the kernel falls back to the
host computation so the returned output is always correct.
"""

import numpy as np

B, TGT, SRC = 512, 49, 40
E, H = 768, 12
HD = E // H
FF = 3072
EPS = 1e-5
NCORES = 8


def _block_np(hidden_states, past_key_values, ln1_g, ln1_b, q_w, q_b, k_w, k_b,
              v_w, v_b, wqk_w, wqk_b, wk_w, wk_b, wb_w, wb_b, wm_w, wm_b,
              trs_w, trs_b, trs2_w, trs2_b, out_w, out_b, ln2_g, ln2_b,
              fc1_w, fc1_b, fc2_w, fc2_b, xp):
    """Reference math on module xp (numpy or jax.numpy), batch-shape agnostic."""
    Bs = hidden_states.shape[0]

    def ln(x, g, b):
        mu = xp.mean(x, axis=-1, keepdims=True)
        var = xp.var(x, axis=-1, keepdims=True)
        return (x - mu) / xp.sqrt(var + EPS) * g + b

    def erf(x):
        if xp is np:
            from scipy.special import erf as _e
            return _e(x)
        import jax
        return jax.scipy.special.erf(x)

    def mm(a, w):
        return a @ w

    residual = hidden_states
    x = ln(hidden_states, ln1_g, ln1_b)

    q = mm(x, q_w) + q_b                               # [B,49,E]
    # trs over seq dim without materializing the [B,E,49] transpose
    q = xp.einsum("bte,ts->bse", q, trs_w) + trs_b[None, :, None]  # [B,40,E]
    q = 0.5 * q * (1.0 + erf(q / np.sqrt(2.0).astype(np.float32)))  # exact gelu

    k = mm(past_key_values, k_w) + k_b                    # [B,40,E]
    v = mm(past_key_values, v_w) + v_b

    # faithful raw reshape to [B, H, SRC, HD] == contiguous regroup
    q = q.reshape(Bs, H, SRC, HD)
    k = k.reshape(Bs, H, SRC, HD)
    v = v.reshape(Bs, H, SRC, HD)

    Bk = (mm(q, wqk_w) + wqk_b) * (mm(k, wk_w) + wk_b)  # [B,H,40,HD]
    b = Bk @ wb_w + wb_b                               # [B,H,40,1]
    m = xp.einsum("bhsd,s->bhd", Bk, wm_w[:, 0])[:, :, None, :] + wm_b  # [B,H,1,HD]

    def softmax(z, axis):
        z = z - xp.max(z, axis=axis, keepdims=True)
        ez = xp.exp(z)
        return ez / xp.sum(ez, axis=axis, keepdims=True)

    attn_w = softmax(b, axis=-2)
    attn_w2 = softmax(m, axis=-1) - np.float32(0.05)

    out = (attn_w + attn_w2) * v                       # [B,H,40,HD]

    out = xp.einsum("bhsd,st->bhtd", out, trs2_w) + trs2_b[None, None, :, None]
    out = xp.swapaxes(out, 1, 2).reshape(Bs, TGT, E)
    out = mm(out, out_w) + out_b

    hs = residual + out
    x = ln(hs, ln2_g, ln2_b)
    h1 = mm(x, fc1_w) + fc1_b
    h1 = h1 * (1.0 / (1.0 + xp.exp(-1.702 * h1)))      # quick_gelu
    x = mm(h1, fc2_w) + fc2_b
    return hs + x


def _block_jax(hidden_states, past_key_values, ln1_g, ln1_b, q_w, q_b, k_w, k_b,
               v_w, v_b, wqk_w, wqk_b, wk_w, wk_b, wb_w, wb_b, wm_w, wm_b,
               trs_w, trs_b, trs2_w, trs2_b, out_w, out_b, ln2_g, ln2_b,
               fc1_w, fc1_b, fc2_w, fc2_b):
    """XLA-friendly reformulation: 2D GEMMs, no per-head batched matmuls,
    no materialized [B,E,49] transposes."""
    import jax
    import jax.numpy as jnp

    Bs = hidden_states.shape[0]
    Rt, Rs = Bs * TGT, Bs * SRC

    def ln(x, g, b):
        mu = jnp.mean(x, axis=-1, keepdims=True)
        var = jnp.var(x, axis=-1, keepdims=True)
        return (x - mu) * jax.lax.rsqrt(var + EPS) * g + b

    bf = jnp.bfloat16

    def bmm(a, w):
        return jnp.matmul(a.astype(bf), w.astype(bf),
                          preferred_element_type=jnp.float32)

    residual = hidden_states                               # [B,49,E]
    x = ln(hidden_states, ln1_g, ln1_b)

    q = (bmm(x.reshape(Rt, E), q_w) + q_b).reshape(Bs, TGT, E)
    # trs over the token dim (49 -> 40)
    q = jnp.einsum("bte,ts->bse", q.astype(bf), trs_w.astype(bf),
                   preferred_element_type=jnp.float32) + trs_b[None, :, None]
    q = 0.5 * q * (1.0 + jax.scipy.special.erf(q * np.float32(1.0 / np.sqrt(2.0))))

    pk2 = past_key_values.reshape(Rs, E)
    k = bmm(pk2, k_w) + k_b                                # [Rs,E]
    v = bmm(pk2, v_w) + v_b

    # faithful raw reshape: [B,40,768] == [B*480, 64] contiguous
    q4 = q.reshape(Bs * H * SRC, HD)
    k4 = k.reshape(Bs * H * SRC, HD)
    v4 = v.reshape(Bs, H, SRC, HD)

    BB = (bmm(q4, wqk_w) + wqk_b) * (bmm(k4, wk_w) + wk_b)  # [B*480, 64]
    bb = (BB @ wb_w + wb_b).reshape(Bs, H, SRC)            # [B,H,40]
    BB4 = BB.reshape(Bs, H, SRC, HD)
    m = (BB4 * wm_w[:, 0][None, None, :, None]).sum(-2) + wm_b  # [B,H,64]

    attn_w = jax.nn.softmax(bb, axis=-1)                   # over 40
    attn_w2 = jax.nn.softmax(m, axis=-1) - np.float32(0.05)  # over 64

    out = (attn_w[..., :, None] + attn_w2[..., None, :]) * v4   # [B,H,40,64]

    out = jnp.einsum("bhsd,st->bhtd", out.astype(bf), trs2_w.astype(bf),
                     preferred_element_type=jnp.float32) \
        + trs2_b[None, None, :, None]
    out = out.transpose(0, 2, 1, 3).reshape(Rt, E)
    out = bmm(out, out_w) + out_b

    hs = residual.reshape(Rt, E) + out
    x = ln(hs, ln2_g, ln2_b)
    h1 = bmm(x, fc1_w) + fc1_b
    h1 = h1 * jax.nn.sigmoid(np.float32(1.702) * h1)
    y = bmm(h1, fc2_w) + fc2_b
    return (hs + y).reshape(Bs, TGT, E)


_ARGS = ["hidden_states", "past_key_values", "ln1_g", "ln1_b", "q_w", "q_b",
         "k_w", "k_b", "v_w", "v_b", "wqk_w", "wqk_b", "wk_w", "wk_b",
         "wb_w", "wb_b", "wm_w", "wm_b", "trs_w", "trs_b", "trs2_w", "trs2_b",
         "out_w", "out_b", "ln2_g", "ln2_b", "fc1_w", "fc1_b", "fc2_w", "fc2_b"]

_cached = {}


def _device_fn():
    """Build (once) the jitted 8-core data-parallel executable."""
    if "fn" in _cached:
        return _cached["fn"]
    import jax
    from jax.sharding import Mesh, PartitionSpec as P
    from jax.experimental.shard_map import shard_map

    devices = jax.devices()[:NCORES]
    assert len(devices) == NCORES
    mesh = Mesh(np.asarray(devices), ("core",))
    in_specs = tuple([P("core"), P("core")] + [P()] * (len(_ARGS) - 2))

    def body(*args):
        return _block_jax(*args)

    fn = jax.jit(shard_map(body, mesh=mesh, in_specs=in_specs,
                           out_specs=P("core"), check_rep=False))
    _cached["fn"] = fn
    return fn


def kernel(**inputs) -> np.ndarray:
    args = [np.asarray(inputs[n], dtype=np.float32) for n in _ARGS]

    result = None
    try:
        fn = _device_fn()
        out = fn(*args)
        result = np.asarray(out, dtype=np.float32)
        if not _cached.get("checked"):
            # host check value: example 0 only (first call only)
            chk = _block_np(*([args[0][:1], args[1][:1]] + args[2:]), xp=np)
            scale = max(float(np.abs(chk).max()), 1e-6)
            rel = float(np.abs(result[:1] - chk).max()) / scale
            if not np.isfinite(rel) or rel > 1.5e-2:
                result = None
            else:
                _cached["checked"] = True
    except Exception:
        result = None

    if result is None:
        # host fallback: always correct
        result = np.asarray(_block_np(*args, xp=np), dtype=np.float32)
    return result


if __name__ == "__main__":
    rng = np.random.default_rng(0)
    ins = {n: rng.standard_normal((1,), dtype=np.float32) for n in _ARGS}
    print("module import ok")

